# revision 53
# baseline (speedup 1.0000x reference)
"""Trainium2 Bass kernel for the DGN message-passing network.

Computation (per batch item b):
    h = relu(x @ enc_w + enc_b)                      [N, H]
    for p in 0..P-1:
        v = relu(h @ wv[p] + bv[p]); q = relu(h @ wq[p] + bq[p]); k = relu(h @ wk[p] + bk[p])
        att = softmax(q @ k.T  masked by mask, axis=-1)
        h = relu((att @ v) @ wo[p] + bo[p])
    y = h @ qw + qb                                  [N, A]

Sharding: data-parallel over the batch dim across 8 NeuronCores (16 items
per core), weights replicated, no cross-core communication.

On-chip layout: everything is kept transposed ([H, N] with H on partitions)
so no transposes are ever required:
  * hT/qT/kT = [H=128, N=512];   projections:  qT = wq.T @ hT  (lhsT = wq)
  * scoresT[m, n] = q[n]·k[m] computed directly as kT-chunk.T @ qT
  * softmax over m (= partition axis of scoresT): exp(s)*mask -> rowsum via
    an all-ones [128,128] matmul (broadcasts the row-sum to all partitions)
    -> 1/rowsum = exp(-ln(rowsum)) (same ACT table set as the softmax Exp,
    no table switching) -> multiply into att@v output.
    No max-subtraction: scores are O(5), exp is safe, softmax shift-invariant.
  * v is built natively in [m, h] layout as one [128, NCH*H] tile: a K=1
    ones-row x (bv tiled 4x) matmul preloads the bias into PSUM in one shot,
    then 4 accumulating h-chunk @ wv matmuls land the projection on top.

Schedule: a skewed software pipeline. Each item walks the 14 macro-steps
    DMA . ENC P0 S0 . A0 O0 P1 S1 . A1 O1 HEAD
and item i starts one step after item i-1, so ~12 items are in flight.
The tensor engine therefore always has independent matmuls to run while an
item's exp/mask/recip chain is on ACT/DVE/GpSimd - without this the PE
duty-cycle drops, the HAM clock-gate throttles it to 1.2 GHz, and every
matmul doubles in cost (measured on the naive per-item schedule). The bare
"." slack ticks are load-bearing: removing the one after DMA measured +33us.

Precision/engine split of the attention tile (chunks = 128-row blocks of m):
the whole exp(scores) tile and v are fp8e4m3, so all rowsum and att@v
contractions run as fp8 DoubleRow matmuls (2 contraction rows per cycle).
The mask multiply is split to balance engines: chunks 0-1 are masked by
GpSimd fp8 multiplies (keeps the otherwise-idle GpSimd loaded; bitwise ops
aren't supported there), chunks 2-3 by a single DVE int32 bitwise-AND over
packed 0xFF/0x00 bytes (4 fp8 lanes per word). Scores never exceed ~4.5
here so exp stays far below fp8e4m3's 448 max, and the softmax normalizer
is computed from the same quantized weights so the quantization largely
cancels (measured end-to-end rel err 4.8e-3, matching a host simulation of
the quantization alone; all-bf16 measured 2.1e-3, tolerance 2e-2).

Engine balance per item (~8.5us each): PE all matmuls; ACT enc-relu + exp +
recip (1/rowsum = exp(-ln(r)), same ACT table set as exp) + 1/4 of v-relus;
DVE q/k/o bias-relus, 3/4 of v-relus, otn normalize, head bias, mask ANDs;
GpSimd fp8 mask mults.
"""

import numpy as np

import concourse.bass as bass
import concourse.mybir as mybir
import concourse.tile as tile
from concourse.bass import ts
from concourse.bass_utils import run_bass_kernel_spmd

F32 = mybir.dt.float32
BF16 = mybir.dt.bfloat16
FP8 = mybir.dt.float8e4
I32 = mybir.dt.int32
AF = mybir.ActivationFunctionType
OP = mybir.AluOpType

N_CORES = 8
B, N, DIN, H, P, A = 128, 512, 64, 128, 2, 16
IPC = B // N_CORES  # batch items per core
NCH = N // 128      # 128-row chunks of the agent dim

# The attention tile is split by chunk-pairs: chunks 0-1 stay bf16 and are
# masked by GpSimd multiplies (bitwise ops aren't supported there); chunks
# 2-3 are fp8 and masked by one cheap DVE int32 bitwise-AND (mask bytes are
# 0xFF / 0x00), then consumed by fp8 DoubleRow rowsum/attv matmuls.


def _spill_excess_waits(nc):
    """Walrus codegen has limited sync-wait slots per instruction: a
    self-loading fp32/fp32r Matmult takes only 1 (waits land on its fused
    LDWEIGHTS micro-op) and sequencer ctrl ops (Drain/NoOp) take 4. Spill
    excess waits onto NoOps inserted just before the instruction on the same
    engine - the engine blocks at the NoOp, so ordering semantics are kept.
    """
    counter = [0]

    def make_nop(engine, waits):
        counter[0] += 1
        nop = mybir.InstNoOp(name=f"I-waitspill-{counter[0]}")
        nop.engine = engine
        nop.sync_info = mybir.SyncInfo(on_wait=list(waits), on_update=[])
        return nop

    def sem_clear_insts(inst):
        """This walrus build rejects EVENT_SEMAPHORE_RANGE_CLEAR ("ISA wrong
        length"); expand Tile's tail range-clear into per-sem writes."""
        first = inst.ant_dict["range_first"]
        last = inst.ant_dict["range_last"]
        res = []
        for s in range(first, last + 1):
            counter[0] += 1
            ev = mybir.InstEventSemaphore(name=f"I-semclear-{counter[0]}")
            ev.engine = inst.engine
            ev.sync_info = mybir.SyncInfo(
                on_wait=list(inst.sync_info.on_wait) if (s == first and inst.sync_info) else [],
                on_update=[mybir.SyncUpdate(
                    sync_type="semaphore", id=s,
                    update_mode="sem-wr-imm", update_value=0,
                )],
            )
            res.append(ev)
        return res

    for fn in nc.m.functions:
        for blk in fn.blocks:
            out = []
            for inst in blk.instructions:
                if (type(inst).__name__ == "InstISA"
                        and inst.ant_dict.get("header", {}).get("opcode") == 176):
                    out.extend(sem_clear_insts(inst))
                    continue
                si = inst.sync_info
                waits = list(si.on_wait) if si is not None else []
                limit = 1
                if len(waits) > limit:
                    keep = waits[-limit:] if limit else []
                    spill = waits[: len(waits) - limit]
                    for w in spill:
                        out.append(make_nop(inst.engine, [w]))
                    inst.sync_info.on_wait = keep
                out.append(inst)
            blk.instructions = out


def build_program():
    nc = bass.Bass("TRN2", target_bir_lowering=False, debug=False)

    xt_d = nc.dram_tensor("xt", [IPC, DIN, N], BF16, kind="ExternalInput").ap()
    # masks pre-laid-out host-side as the SBUF images: chunks 0-1 as fp8
    # multiplicands (1.0/0.0), chunks 2-3 as AND-words (0xFF/0x00 as int32)
    mf8_d = nc.dram_tensor("maskf8", [IPC, 128, 2 * N], FP8, kind="ExternalInput").ap()
    m32_d = nc.dram_tensor("mask32", [IPC, 128, 2 * N // 4], I32, kind="ExternalInput").ap()
    # all [H,H] weight matrices packed: (wq,wk,wv,wo) x P along the free axis
    wcat_d = nc.dram_tensor("wcat", [H, 4 * P * H], BF16, kind="ExternalInput").ap()
    # biases packed: encb, (bq,bk,bo) x P
    bcat_d = nc.dram_tensor("bcat", [H, 1 + 3 * P], F32, kind="ExternalInput").ap()
    encw_d = nc.dram_tensor("enc_w", [DIN, H], BF16, kind="ExternalInput").ap()
    bvr_d = nc.dram_tensor("bvr", [1, P * N], BF16, kind="ExternalInput").ap()
    qw_d = nc.dram_tensor("qw", [H, A], BF16, kind="ExternalInput").ap()
    ones_d = nc.dram_tensor("ones", [128, 128], BF16, kind="ExternalInput").ap()
    ones8_d = nc.dram_tensor("ones8", [128, 2, H], FP8, kind="ExternalInput").ap()
    qb_d = nc.dram_tensor("qb", [A, 1], F32, kind="ExternalInput").ap()
    yt_d = nc.dram_tensor("yt", [IPC, A, N], F32, kind="ExternalOutput").ap()

    with tile.TileContext(nc) as tc:
        with (
            tc.tile_pool(name="weights", bufs=1) as wpool,
            tc.tile_pool(name="xin", bufs=4) as xpool,
            tc.tile_pool(name="maskin", bufs=12) as mpool,
            tc.tile_pool(name="hbuf", bufs=7) as hpool,
            tc.tile_pool(name="qbuf", bufs=3) as qpool,
            tc.tile_pool(name="kbuf", bufs=3) as kpool,
            tc.tile_pool(name="vbuf", bufs=5) as vpool,
            tc.tile_pool(name="pbuf", bufs=4) as ppool,
            tc.tile_pool(name="rbuf", bufs=3) as rpool,
            tc.tile_pool(name="obuf", bufs=3) as opool,
            tc.tile_pool(name="ybuf", bufs=3) as ypool,
            tc.tile_pool(name="psum", bufs=2, space="PSUM") as psum,
        ):
            st = [dict() for _ in range(IPC)]  # per-item live tiles

            def st_dma(i):
                if "xt" in st[i]:
                    return
                xt_t = xpool.tile([DIN, N], BF16, tag="xt")
                nc.sync.dma_start(out=xt_t[:], in_=xt_d[i])
                mf8_t = mpool.tile([128, 2 * N], FP8, tag="mb")
                nc.sync.dma_start(out=mf8_t[:], in_=mf8_d[i])
                m32_t = mpool.tile([128, 2 * N // 4], I32, tag="m32")
                nc.sync.dma_start(out=m32_t[:], in_=m32_d[i])
                st[i]["xt"], st[i]["mb"], st[i]["m32"] = xt_t, mf8_t, m32_t

            # ---- input for item 0 first (unblocks the pipeline), then the
            # packed weights (7 DMAs), then everything else ----
            st_dma(0)

            encw_t = wpool.tile([DIN, H], BF16, tag="encw")
            nc.sync.dma_start(out=encw_t[:], in_=encw_d[:])
            bcat_t = wpool.tile([H, 1 + 3 * P], F32, tag="bcat")
            nc.sync.dma_start(out=bcat_t[:], in_=bcat_d[:])
            ones_t = wpool.tile([128, 128], BF16, tag="ones")
            nc.sync.dma_start(out=ones_t[:], in_=ones_d[:])
            ones8_t = wpool.tile([128, 2, H], FP8, tag="ones8")
            nc.sync.dma_start(out=ones8_t[:], in_=ones8_d[:])
            # pre-warm the ACT exp table set while weights stream in
            warm_t = wpool.tile([1, 1], F32, tag="warm")
            nc.scalar.activation(warm_t[:], ones_t[0:1, 0:1], AF.Exp)
            # keep the PE clock-gate (HAM) warm across the DMA-bound preamble
            wrm = psum.tile([128, NCH * N], F32, tag="sc", bufs=1)
            for _ in range(24):
                nc.tensor.matmul(wrm[:, 0:128], lhsT=(ones_t[:]), rhs=(ones_t[:]),
                                 start=True, stop=True)
            wcat_t = wpool.tile([H, 4 * P * H], BF16, tag="wcat")
            nc.sync.dma_start(out=wcat_t[:], in_=wcat_d[:])
            bvr_t2 = wpool.tile([1, P * N], BF16, tag="bvr")
            nc.sync.dma_start(out=bvr_t2[:], in_=bvr_d[:])
            qw_t = wpool.tile([H, A], BF16, tag="qw")
            nc.sync.dma_start(out=qw_t[:], in_=qw_d[:])
            qb_t = wpool.tile([A, 1], F32, tag="qb")
            nc.sync.dma_start(out=qb_t[:], in_=qb_d[:])

            encb_t = bcat_t[:, 0:1]
            wq_t = [wcat_t[:, ts(4 * p + 0, H)] for p in range(P)]
            wk_t = [wcat_t[:, ts(4 * p + 1, H)] for p in range(P)]
            wv_t = [wcat_t[:, ts(4 * p + 2, H)] for p in range(P)]
            wo_t = [wcat_t[:, ts(4 * p + 3, H)] for p in range(P)]
            bq_t = [bcat_t[:, 1 + 3 * p + 0:2 + 3 * p] for p in range(P)]
            bk_t = [bcat_t[:, 2 + 3 * p:3 + 3 * p] for p in range(P)]
            bo_t = [bcat_t[:, 3 + 3 * p:4 + 3 * p] for p in range(P)]
            bvr_t = [bvr_t2[:, ts(p, N)] for p in range(P)]

            def st_enc(i):
                hp = psum.tile([H, N], F32, tag="proj")
                nc.tensor.matmul(hp[:], lhsT=(encw_t[:]), rhs=(st[i]["xt"][:]),
                                 start=True, stop=True)
                hT = hpool.tile([H, N], BF16, tag="h")
                nc.scalar.activation(hT[:], hp[:], AF.Relu, bias=encb_t[:])
                st[i]["h"] = hT

            def st_proj(i, p):
                hT = st[i]["h"]
                qp = psum.tile([H, N], F32, tag="proj")
                nc.tensor.matmul(qp[:], lhsT=(wq_t[p][:]), rhs=(hT[:]), start=True, stop=True)
                qT = qpool.tile([H, N], BF16, tag="q")
                nc.vector.tensor_scalar(
                    out=qT[:], in0=qp[:], scalar1=bq_t[p][:], scalar2=0.0,
                    op0=OP.add, op1=OP.max,
                )
                kp = psum.tile([H, N], F32, tag="proj")
                nc.tensor.matmul(kp[:], lhsT=(wk_t[p][:]), rhs=(hT[:]), start=True, stop=True)
                kT = kpool.tile([H, N], BF16, tag="k")
                nc.vector.tensor_scalar(
                    out=kT[:], in0=kp[:], scalar1=bk_t[p][:], scalar2=0.0,
                    op0=OP.add, op1=OP.max,
                )
                # v in natural [m, h] layout: one bias preload + 4 chunk matmuls.
                # chunks 0-1 land in bf16, chunks 2-3 in fp8 (DoubleRow operand).
                vp = psum.tile([128, N], F32, tag="proj")
                nc.tensor.matmul(vp[:], lhsT=(ones_t[0:1, :]), rhs=(bvr_t[p][:]),
                                 start=True, stop=False)
                for c in range(NCH):
                    nc.tensor.matmul(vp[:, ts(c, H)], lhsT=(hT[:, ts(c, 128)]),
                                     rhs=(wv_t[p][:]), start=False, stop=True)
                vn = vpool.tile([128, NCH * H], FP8, tag="v")
                if i % 4 == 0:  # shed ~1/4 of the v-relus to ACT (DVE is the
                    nc.scalar.activation(vn[:], vp[:], AF.Relu)  # top engine)
                else:
                    nc.vector.tensor_scalar_max(vn[:], vp[:], 0.0)
                st[i]["q"], st[i]["k"], st[i]["v"] = qT, kT, vn

            def st_scores(i, p):
                qT, kT = st[i]["q"], st[i]["k"]
                # all 4 score chunks into one PSUM tile, one big exp, then
                # masks: chunks 0-1 via GpSimd fp8 multiplies, chunks 2-3 via
                # one DVE int32 bitwise-AND
                pT = ppool.tile([128, NCH * N], FP8, tag="p")
                scp = psum.tile([128, NCH * N], F32, tag="sc", bufs=1)
                for c in range(NCH):
                    nc.tensor.matmul(
                        scp[:, ts(c, N)], lhsT=(kT[:, ts(c, 128)]), rhs=(qT[:]),
                        start=True, stop=True,
                    )
                nc.scalar.activation(pT[:], scp[:], AF.Exp)
                for cc in range(2):
                    nc.gpsimd.tensor_tensor(
                        out=pT[:, ts(cc, N)], in0=pT[:, ts(cc, N)],
                        in1=st[i]["mb"][:, ts(cc, N)], op=OP.mult,
                    )
                p32 = pT.bitcast(I32)
                nc.vector.tensor_tensor(
                    out=p32[:, 2 * N // 4:], in0=p32[:, 2 * N // 4:],
                    in1=st[i]["m32"][:], op=OP.bitwise_and,
                )
                st[i]["p"] = pT

            def st_attn(i, p):
                pT, vn = st[i]["p"], st[i]["v"]
                pT3 = pT.rearrange("p (k d) -> p k d", d=N)  # [128, NCH, N]
                pa3, pb3 = pT3[:, 0:2, :], pT3[:, 2:4, :]
                vn3 = vn.rearrange("p (k d) -> p k d", d=H)  # [128, NCH, H]
                # rowsum over m: two fp8 DoubleRow matmuls, pair B first (its
                # AND mask is faster than pair A's GpSimd multiplies)
                rst = psum.tile([128, N], F32, tag="rs", bufs=1)
                nc.tensor.matmul(rst[:], lhsT=(ones8_t[:]), rhs=(pb3),
                                 start=True, stop=False,
                                 perf_mode=mybir.MatmulPerfMode.DoubleRow)
                nc.tensor.matmul(rst[:], lhsT=(ones8_t[:]), rhs=(pa3),
                                 start=False, stop=True,
                                 perf_mode=mybir.MatmulPerfMode.DoubleRow)
                lnr = rpool.tile([H, N], F32, tag="lnr")
                nc.scalar.activation(lnr[:], rst[:], AF.Ln)
                ot = psum.tile([H, N], F32, tag="ot", bufs=1)
                nc.tensor.matmul(ot[:], lhsT=(vn3[:, 2:4, :]), rhs=(pb3),
                                 start=True, stop=False,
                                 perf_mode=mybir.MatmulPerfMode.DoubleRow)
                nc.tensor.matmul(ot[:], lhsT=(vn3[:, 0:2, :]), rhs=(pa3),
                                 start=False, stop=True,
                                 perf_mode=mybir.MatmulPerfMode.DoubleRow)
                recipb = rpool.tile([H, N], F32, tag="recip")
                nc.scalar.activation(recipb[:], lnr[:], AF.Exp, scale=-1.0)
                otn = opool.tile([H, N], BF16, tag="otn")
                nc.vector.tensor_tensor(out=otn[:], in0=ot[:], in1=recipb[:], op=OP.mult)
                st[i]["otn"] = otn

            def st_out(i, p):
                h2p = psum.tile([H, N], F32, tag="proj")
                nc.tensor.matmul(h2p[:], lhsT=(wo_t[p][:]), rhs=(st[i]["otn"][:]),
                                 start=True, stop=True)
                hT = hpool.tile([H, N], BF16, tag="h")
                nc.vector.tensor_scalar(
                    out=hT[:], in0=h2p[:], scalar1=bo_t[p][:], scalar2=0.0,
                    op0=OP.add, op1=OP.max,
                )
                st[i]["h"] = hT

            def st_head(i):
                yp = psum.tile([H, N], F32, tag="proj")
                nc.tensor.matmul(yp[0:A, :], lhsT=(qw_t[:]), rhs=(st[i]["h"][:]),
                                 start=True, stop=True)
                yt_t = ypool.tile([A, N], F32, tag="y")
                nc.vector.tensor_scalar_add(yt_t[:], yp[0:A, :], qb_t[:])
                nc.sync.dma_start(out=yt_d[i], in_=yt_t[:])
                st[i].clear()

            steps = [
                st_dma,
                lambda i: None,
                st_enc,
                lambda i: st_proj(i, 0),
                lambda i: st_scores(i, 0),
                lambda i: None,
                lambda i: st_attn(i, 0),
                lambda i: st_out(i, 0),
                lambda i: st_proj(i, 1),
                lambda i: st_scores(i, 1),
                lambda i: None,
                lambda i: st_attn(i, 1),
                lambda i: st_out(i, 1),
                st_head,
            ]
            n_steps = len(steps)
            for t in range(IPC + n_steps - 1):
                for i in range(max(0, t - n_steps + 1), min(t, IPC - 1) + 1):
                    steps[t - i](i)

    _spill_excess_waits(nc)
    return nc


_prog_cache = None


def _get_program():
    global _prog_cache
    if _prog_cache is None:
        _prog_cache = build_program()
    return _prog_cache


def _make_in_maps(x, mask, enc_w, enc_b, wv, bv, wk, bk, wq, bq, wo, bo, qw, qb):
    import ml_dtypes
    bf = lambda a: np.ascontiguousarray(np.asarray(a, dtype=np.float32).astype(ml_dtypes.bfloat16))
    f = lambda a: np.ascontiguousarray(np.asarray(a, dtype=np.float32))
    x, mask = f(x), f(mask)
    wq, wk, wv, wo = bf(wq), bf(wk), bf(wv), bf(wo)
    bq, bk, bo = f(bq), f(bk), f(bo)
    wcat = np.concatenate(
        [m for p in range(P) for m in (wq[p], wk[p], wv[p], wo[p])], axis=1)
    bcat = np.stack(
        [f(enc_b)] + [b for p in range(P) for b in (bq[p], bk[p], bo[p])], axis=1)
    shared = {
        "enc_w": bf(enc_w),
        "wcat": np.ascontiguousarray(wcat),
        "bcat": np.ascontiguousarray(bcat),
        "bvr": np.ascontiguousarray(np.tile(bf(bv), (1, NCH)).reshape(1, P * N)),
        "qw": bf(qw),
        "ones": np.ones((128, 128), dtype=ml_dtypes.bfloat16),
        "ones8": np.ones((128, 2, H), dtype=ml_dtypes.float8_e4m3fn),
        "qb": f(qb).reshape(A, 1),
    }
    in_maps = []
    for c in range(N_CORES):
        sl = slice(c * IPC, (c + 1) * IPC)
        mskt = (mask[sl].transpose(0, 2, 1).reshape(IPC, NCH, 128, N)
                .transpose(0, 2, 1, 3).reshape(IPC, 128, NCH * N))
        mf8 = mskt[:, :, :2 * N].astype(ml_dtypes.float8_e4m3fn)
        mbytes = np.where(mskt[:, :, 2 * N:] > 0.5, np.uint8(0xFF), np.uint8(0))
        m32 = np.ascontiguousarray(mbytes).view(np.int32)
        in_maps.append({
            "xt": np.ascontiguousarray(x[sl].transpose(0, 2, 1).astype(ml_dtypes.bfloat16)),
            "maskf8": np.ascontiguousarray(mf8),
            "mask32": np.ascontiguousarray(m32),
            **shared,
        })
    return in_maps


def run(trace=False, tmpdir=None, **inputs):
    nc = _get_program()
    in_maps = _make_in_maps(**inputs)
    res = run_bass_kernel_spmd(nc, in_maps, list(range(N_CORES)), trace=trace,
                               tmpdir=tmpdir)
    y = np.concatenate(
        [r["yt"].transpose(0, 2, 1) for r in res.results], axis=0
    ).astype(np.float32)
    return y, res


def kernel(**inputs):
    y, _ = run(trace=False, **inputs)
    return y
